# revision 16
# baseline (speedup 1.0000x reference)
"""L2 self-attention (q==k) Bass/Tile kernel for 8 TRN2 NeuronCores.

Sharding: core c = 2*b + g handles batch b and head-group g (8 of 16 heads).
Each core computes the partial output  attn_out_bg @ Wo[g*512:(g+1)*512, :].
Host sums the two partials per batch and adds bo.

Math per head (s = DIM_HEAD**-0.5):
  P_ij = exp(2s*AB_ij - s*AA_j) / sum_j exp(2s*AB_ij - s*AA_j)
  out = P @ v ; den folded in as a 65th (ones) column of v.

v3: single fused pipeline. Attention runs head-pair units (the two 64-row
S matmuls of a dt-tile execute concurrently in opposite PE row-half
quadrants); projection / AA / output-projection matmuls interleave into
the attention stream as transient allocations from the same PSUM ring so
the scalar engine (exp) paces the whole kernel. Normalization reciprocals
are deferred one unit so their DVE time never blocks the ring-chunk
copies.
"""

import numpy as np

B, N, D = 4, 2048, 1024
HEADS, DIM_HEAD = 16, 64
INNER = HEADS * DIM_HEAD
SCALE = DIM_HEAD ** -0.5

NCORES = 8
NH = 8            # heads per core
DL = NH * DIM_HEAD  # 512 local inner dims
KT = D // 128     # 8 full contraction tiles for projections
JT = N // 128     # 16 key tiles
IC = 2            # i-chunks of 1024 columns
ICW = N // IC     # 1024

_CACHE = {}


def _build_nc():
    import concourse.bacc as bacc
    import concourse.bass as bass
    import concourse.mybir as mybir
    import concourse.tile as tile
    from concourse.masks import make_identity

    f32 = mybir.dt.float32
    bf16 = mybir.dt.bfloat16
    X = mybir.AxisListType.X
    EXP = mybir.ActivationFunctionType.Exp

    nc = bacc.Bacc("TRN2", target_bir_lowering=False, debug=False,
                   num_devices=NCORES)

    xTa_d = nc.dram_tensor("xTa", [D + 1, N], bf16, kind="ExternalInput")
    wqa_d = nc.dram_tensor("wqa", [D + 1, DL], bf16, kind="ExternalInput")
    wva_d = nc.dram_tensor("wva", [D + 1, DL], bf16, kind="ExternalInput")
    wo_d = nc.dram_tensor("wo", [DL, D], bf16, kind="ExternalInput")
    part_d = nc.dram_tensor("part", [N, D], bf16, kind="ExternalOutput")
    xTa = xTa_d.ap()
    wqa = wqa_d.ap()
    wva = wva_d.ap()
    wo_ap = wo_d.ap()
    part = part_d.ap()

    with tile.TileContext(nc) as tc, \
         tc.tile_pool(name="persist", bufs=1) as persist, \
         tc.tile_pool(name="gp", bufs=6) as gp, \
         tc.tile_pool(name="nrm", bufs=2) as nrm, \
         tc.tile_pool(name="osb", bufs=3) as osb, \
         tc.tile_pool(name="ring", bufs=2, space="PSUM") as ring, \
         tc.tile_pool(name="nps", bufs=2, space="PSUM") as nps:

        # ---- persistent tensors ----
        qT = [persist.tile([128, N], bf16, tag=f"qT{t}", name=f"qT{t}")
              for t in range(4)]
        v_aug = persist.tile([128, JT, NH * 65], bf16, tag="v_aug", name="v_aug")
        aa = persist.tile([128, NH * JT], f32, tag="aa", name="aa")
        ident4 = persist.tile([128, 4, 128], f32, tag="id4", name="ident4")
        ones1 = persist.tile([128, JT, 1], bf16, tag="ones1", name="ones1")
        xt = [persist.tile([128, N], bf16, tag=f"xt{k}", name=f"xt{k}")
              for k in range(KT)]
        xt.append(persist.tile([1, N], bf16, tag="xt_ones", name="xt_ones"))
        wq = [persist.tile([128, DL], bf16, tag=f"wq{k}", name=f"wq{k}")
              for k in range(KT)]
        wq.append(persist.tile([1, DL], bf16, tag="wq_b", name="wq_b"))
        wv = [persist.tile([128, DL], bf16, tag=f"wv{k}", name=f"wv{k}")
              for k in range(KT)]
        wv.append(persist.tile([1, DL], bf16, tag="wv_b", name="wv_b"))
        ot = [persist.tile([128, N], bf16, tag=f"ot{t}", name=f"ot{t}")
              for t in range(4)]
        wo_sb = [persist.tile([128, D], bf16, tag=f"wo{t}", name=f"wo{t}")
                 for t in range(4)]

        for c in range(4):
            make_identity(nc, ident4[:, c, :])
        nc.vector.tensor_scalar_mul(ident4, ident4, -SCALE)
        nc.vector.memset(ones1, 1.0)
        for h in range(NH):
            nc.vector.tensor_copy(v_aug[:, :, h * 65 + 64 : h * 65 + 65], ones1)

        for k in range(KT):
            sl = slice(k * 128, (k + 1) * 128)
            nc.sync.dma_start(out=xt[k], in_=xTa[sl, :])
            nc.sync.dma_start(out=wq[k], in_=wqa[sl, :])
            nc.sync.dma_start(out=wv[k], in_=wva[sl, :])
        nc.sync.dma_start(out=xt[KT], in_=xTa[D : D + 1, :])
        nc.sync.dma_start(out=wq[KT], in_=wqa[D : D + 1, :])
        nc.sync.dma_start(out=wv[KT], in_=wva[D : D + 1, :])
        for t in range(4):
            nc.sync.dma_start(out=wo_sb[t], in_=wo_ap[t * 128 : (t + 1) * 128, :])

        # ---- chunk emitters (each = one transient ring allocation) ----
        def qproj_chunk(dt, nck):
            ps = ring.tile([128, ICW], f32, tag="scores", name="ring")
            dsl = slice(dt * 128, (dt + 1) * 128)
            nsl = slice(nck * 512, (nck + 1) * 512)
            for k in range(KT + 1):
                nc.tensor.matmul(ps[:, 0:512], lhsT=wq[k][:, dsl],
                                 rhs=xt[k][:, nsl],
                                 start=(k == 0), stop=(k == KT))
            nc.vector.tensor_copy(qT[dt][:, nsl], ps[:, 0:512])

        def vproj_chunk(it):
            ps = ring.tile([128, ICW], f32, tag="scores", name="ring")
            isl = slice(it * 128, (it + 1) * 128)
            for k in range(KT + 1):
                nc.tensor.matmul(ps[:, 0:512], lhsT=xt[k][:, isl], rhs=wv[k],
                                 start=(k == 0), stop=(k == KT))
            src = ps[:, 0:512].rearrange("p (h w) -> p h w", w=64)
            dst = v_aug[:, it, :].rearrange("p (h w) -> p h w", w=65)
            nc.vector.tensor_copy(dst[:, :, 0:64], src)

        def aa_chunk(hp, half, jq):
            # grams for 4 consecutive jt of head 2*hp+half; diag extract
            dt = hp
            h = 2 * hp + half
            rows = slice(half * 64, half * 64 + 64)
            ps = ring.tile([128, ICW], f32, tag="scores", name="ring")
            for c in range(4):
                jt = jq * 4 + c
                jsl = slice(jt * 128, (jt + 1) * 128)
                nc.tensor.matmul(ps[:, c * 128 : (c + 1) * 128],
                                 lhsT=qT[dt][rows, jsl], rhs=qT[dt][rows, jsl],
                                 start=True, stop=True)
            sc = nrm.tile([128, 4, 128], f32, tag="aasc", name="aasc")
            nc.vector.tensor_mul(sc, ps[:, 0:512].rearrange("p (c w) -> p c w", w=128),
                                 ident4)
            col = h * JT + jq * 4
            nc.vector.reduce_sum(out=aa[:, col : col + 4], in_=sc, axis=X)

        def oproj_chunk(it):
            ps = ring.tile([128, ICW], f32, tag="scores", name="ring")
            isl = slice(it * 128, (it + 1) * 128)
            for ock in range(2):
                osl = slice(ock * 512, (ock + 1) * 512)
                for dlt in range(4):
                    nc.tensor.matmul(ps[:, osl], lhsT=ot[dlt][:, isl],
                                     rhs=wo_sb[dlt][:, osl],
                                     start=(dlt == 0), stop=(dlt == 3))
            ob = osb.tile([128, 1024], bf16, tag="obuf", name="obuf")
            nc.vector.tensor_copy(ob, ps)
            nc.sync.dma_start(out=part[isl, :], in_=ob)

        def norm_copy(nm):
            # at unit end: free the num psum quickly (DVE, bf16 out)
            nmc = []
            for half in range(2):
                t = nrm.tile([65, ICW], bf16, tag="nmc", name="nmc")
                nc.vector.tensor_copy(t, nm[half])
                nmc.append(t)
            return nmc

        def norm_finish(hp, ic, nmc):
            # deferred: reciprocal of the dens, broadcast, scale -> ot
            dt = hp
            i0 = ic * ICW
            for half in range(2):
                rows = slice(half * 64, half * 64 + 64)
                rd = nrm.tile([1, ICW], bf16, tag="rden", name="rden")
                with nc.allow_low_precision(reason="bf16 reciprocal of softmax "
                                            "denominator; 0.4% rel err ok"):
                    nc.vector.reciprocal(rd, nmc[half][64:65, :])
                rdb = nrm.tile([64, ICW], bf16, tag="rdenb", name="rdenb")
                nc.gpsimd.partition_broadcast(rdb, rd)
                nc.vector.tensor_mul(ot[dt][rows, i0 : i0 + ICW],
                                     nmc[half][0:64, :], rdb)

        # ---- extras schedule: units are u = hp*2 + ic (hp outer) ----
        leadin = [lambda nck=nck: qproj_chunk(0, nck) for nck in range(4)]
        leadin += [lambda half=half, jq=jq: aa_chunk(0, half, jq)
                   for half in range(2) for jq in range(4)]
        leadin += [lambda it=it: vproj_chunk(it) for it in range(7)]

        extras = {u: [] for u in range(8)}
        extras[0] = [lambda it=it: vproj_chunk(it) for it in range(7, JT)]
        for k in range(3):
            lst = [lambda nck=nck, d=k + 1: qproj_chunk(d, nck)
                   for nck in range(4)]
            lst += [lambda half=half, p=k + 1: aa_chunk(p, half, 0)
                    for half in range(2)]
            extras[1 + 2 * k] = lst
            extras[2 + 2 * k] = [lambda half=half, jq=jq, p=k + 1: aa_chunk(p, half, jq)
                                 for jq in range(1, 4) for half in range(2)]
        extras[7] = [lambda it=it: oproj_chunk(it) for it in range(8)]

        for f in leadin:
            f()

        # ---- fused attention pipeline ----
        pending = None
        for u in range(8):
            hp, ic = divmod(u, 2)
            dt = hp
            hs = (2 * hp, 2 * hp + 1)
            i0 = ic * ICW
            ex = list(extras[u])
            nm = [nps.tile([65, ICW], f32, tag="num", name=f"num{half}")
                  for half in range(2)]
            gs = [[None] * JT for _ in range(2)]
            for jt in range(JT):
                jsl = slice(jt * 128, (jt + 1) * 128)
                sp = []
                for half in range(2):
                    rows = slice(half * 64, half * 64 + 64)
                    s = ring.tile([128, ICW], f32, tag="scores", name="ring")
                    for q in range(2):
                        qsl = slice(q * 512, (q + 1) * 512)
                        nc.tensor.matmul(
                            s[:, qsl], lhsT=qT[dt][rows, jsl],
                            rhs=qT[dt][rows, i0 + q * 512 : i0 + (q + 1) * 512],
                            start=True, stop=True)
                    sp.append(s)
                for half in range(2):
                    g = gp.tile([128, ICW], bf16, tag="gtile")
                    col = hs[half] * JT + jt
                    nc.scalar.activation(out=g, in_=sp[half], func=EXP,
                                         bias=aa[:, col : col + 1],
                                         scale=2.0 * SCALE)
                    gs[half][jt] = g
                if jt > 0:
                    for half in range(2):
                        vsl = slice(hs[half] * 65, (hs[half] + 1) * 65)
                        for q in range(2):
                            qsl = slice(q * 512, (q + 1) * 512)
                            nc.tensor.matmul(
                                nm[half][:, qsl],
                                lhsT=v_aug[:, jt - 1, vsl],
                                rhs=gs[half][jt - 1][:, qsl],
                                start=(jt == 1), stop=False)
                # deferred normalize of the previous unit: early in unit 7
                # (oproj depends on it), otherwise late so its DVE time
                # runs after this unit's chunk copies
                if pending is not None and jt == (1 if u == 7 else 13):
                    norm_finish(*pending)
                    pending = None
                # drain one extras chunk per step (jt 0 reserved for S ramp)
                if jt >= 1 and ex and not (u == 7 and jt < 5):
                    ex.pop(0)()
            for half in range(2):
                vsl = slice(hs[half] * 65, (hs[half] + 1) * 65)
                for q in range(2):
                    qsl = slice(q * 512, (q + 1) * 512)
                    nc.tensor.matmul(nm[half][:, qsl],
                                     lhsT=v_aug[:, JT - 1, vsl],
                                     rhs=gs[half][JT - 1][:, qsl],
                                     start=False, stop=True)
            for f in ex:
                f()
            nmc = norm_copy(nm)
            pending = (hp, ic, nmc)

        # ---- tail: last normalize + second-half output projection ----
        norm_finish(*pending)
        for it in range(8, JT):
            oproj_chunk(it)

    nc.compile()
    return nc


def _get_nc():
    if "nc" not in _CACHE:
        _CACHE["nc"] = _build_nc()
    return _CACHE["nc"]


def make_in_maps(x, Wq, bq, Wv, bv, Wo, bo):
    import ml_dtypes
    bf16 = ml_dtypes.bfloat16
    x = np.asarray(x, dtype=np.float32)
    Wq = np.asarray(Wq, dtype=np.float32)
    bq = np.asarray(bq, dtype=np.float32)
    Wv = np.asarray(Wv, dtype=np.float32)
    bv = np.asarray(bv, dtype=np.float32)
    Wo = np.asarray(Wo, dtype=np.float32)
    in_maps = []
    for c in range(NCORES):
        b, g = divmod(c, 2)
        gsl = slice(g * DL, (g + 1) * DL)
        xTa = np.concatenate([np.ascontiguousarray(x[b].T),
                              np.ones((1, N), np.float32)], axis=0)
        wqa = np.concatenate([Wq[:, gsl], bq[gsl][None, :]], axis=0)
        wva = np.concatenate([Wv[:, gsl], bv[gsl][None, :]], axis=0)
        in_maps.append({
            "xTa": np.ascontiguousarray(xTa).astype(bf16),
            "wqa": np.ascontiguousarray(wqa).astype(bf16),
            "wva": np.ascontiguousarray(wva).astype(bf16),
            "wo": np.ascontiguousarray(Wo[gsl, :]).astype(bf16),
        })
    return in_maps


def combine_parts(parts, bo):
    bo = np.asarray(bo, dtype=np.float32)
    out = np.empty((B, N, D), np.float32)
    for b in range(B):
        out[b] = np.asarray(parts[2 * b], np.float32) \
            + np.asarray(parts[2 * b + 1], np.float32) + bo
    return out


def kernel(x, Wq, bq, Wv, bv, Wo, bo):
    from concourse.bass_utils import run_bass_kernel_spmd

    nc = _get_nc()
    in_maps = make_in_maps(x, Wq, bq, Wv, bv, Wo, bo)
    res = run_bass_kernel_spmd(nc, in_maps, core_ids=list(range(NCORES)))
    parts = [r["part"] for r in res.results]
    return combine_parts(parts, bo)


# revision 19
# speedup vs baseline: 1.2172x; 1.2172x over previous
"""L2 self-attention (q==k) Bass/Tile kernel for 8 TRN2 NeuronCores.

Sharding: core c = 2*b + g handles batch b and head-group g (8 of 16 heads).
Each core computes the partial output  attn_out_bg @ Wo[g*512:(g+1)*512, :].
Host sums the two partials per batch and adds bo.

Math per head (s = DIM_HEAD**-0.5):
  sim_ij = -s*||q_i - q_j||^2 = 2s*AB_ij - s*AA_i - s*AA_j
  softmax rows are invariant to the per-row constant  -s*AA_i, so
  P_ij = exp(2s*AB_ij - s*AA_j) / sum_j exp(2s*AB_ij - s*AA_j)
  Exponent is <= 0 + bounded (2ab <= a^2+b^2), no max-subtraction needed.
  out = P @ v ; den folded in as a 65th (ones) column of v.

v2: bf16 operands everywhere (halves DMA + enables FWL weight loads),
head-pair interleaving so the two 64-row S matmuls of a dt-tile run
concurrently in opposite PE row-halves, fused AA diag reduction, fast
reciprocal, and output DMA'd straight from PSUM.
"""

import numpy as np

B, N, D = 4, 2048, 1024
HEADS, DIM_HEAD = 16, 64
INNER = HEADS * DIM_HEAD
SCALE = DIM_HEAD ** -0.5

NCORES = 8
NH = 8            # heads per core
DL = NH * DIM_HEAD  # 512 local inner dims
KT = D // 128     # 8 full contraction tiles for projections
JT = N // 128     # 16 key tiles
IC = 2            # i-chunks of 1024 columns
ICW = N // IC     # 1024

_CACHE = {}


def _build_nc():
    import concourse.bacc as bacc
    import concourse.bass as bass
    import concourse.mybir as mybir
    import concourse.tile as tile
    from concourse.masks import make_identity

    f32 = mybir.dt.float32
    bf16 = mybir.dt.bfloat16
    X = mybir.AxisListType.X
    EXP = mybir.ActivationFunctionType.Exp
    MULT = mybir.AluOpType.mult
    ADD = mybir.AluOpType.add

    nc = bacc.Bacc("TRN2", target_bir_lowering=False, debug=False,
                   num_devices=NCORES)

    xTa_d = nc.dram_tensor("xTa", [D + 1, N], bf16, kind="ExternalInput")
    wqa_d = nc.dram_tensor("wqa", [D + 1, DL], bf16, kind="ExternalInput")
    wva_d = nc.dram_tensor("wva", [D + 1, DL], bf16, kind="ExternalInput")
    wo_d = nc.dram_tensor("wo", [DL, D], bf16, kind="ExternalInput")
    part_d = nc.dram_tensor("part", [N, D], bf16, kind="ExternalOutput")
    xTa = xTa_d.ap()
    wqa = wqa_d.ap()
    wva = wva_d.ap()
    wo_ap = wo_d.ap()
    part = part_d.ap()

    with tile.TileContext(nc) as tc, \
         tc.tile_pool(name="persist", bufs=1) as persist:
        # ---- persistent tensors (whole-kernel lifetime) ----
        qT = [persist.tile([128, N], bf16, tag=f"qT{t}", name=f"qT{t}")
              for t in range(4)]
        v_aug = persist.tile([128, JT, NH * 65], bf16, tag="v_aug", name="v_aug")
        aa = persist.tile([128, NH * JT], f32, tag="aa", name="aa")
        ident4 = persist.tile([128, 4, 128], f32, tag="id4", name="ident4")

        ones1 = persist.tile([128, JT, 1], bf16, tag="ones1", name="ones1")
        for c in range(4):
            make_identity(nc, ident4[:, c, :])
        nc.vector.tensor_scalar_mul(ident4, ident4, -SCALE)
        nc.vector.memset(ones1, 1.0)
        for h in range(NH):
            nc.vector.tensor_copy(v_aug[:, :, h * 65 + 64 : h * 65 + 65], ones1)

        # ---- phase 1: projections ----
        with tc.tile_pool(name="pin", bufs=1) as pin:
            xt = [pin.tile([128, N], bf16, tag=f"xt{k}", name=f"xt{k}") for k in range(KT)]
            xt.append(pin.tile([1, N], bf16, tag="xt_ones", name="xt_ones"))
            wq = [pin.tile([128, DL], bf16, tag=f"wq{k}", name=f"wq{k}") for k in range(KT)]
            wq.append(pin.tile([1, DL], bf16, tag="wq_b", name="wq_b"))
            wv = [pin.tile([128, DL], bf16, tag=f"wv{k}", name=f"wv{k}") for k in range(KT)]
            wv.append(pin.tile([1, DL], bf16, tag="wv_b", name="wv_b"))
            for k in range(KT):
                sl = slice(k * 128, (k + 1) * 128)
                nc.sync.dma_start(out=xt[k], in_=xTa[sl, :])
                nc.sync.dma_start(out=wq[k], in_=wqa[sl, :])
                nc.sync.dma_start(out=wv[k], in_=wva[sl, :])
            nc.sync.dma_start(out=xt[KT], in_=xTa[D : D + 1, :])
            nc.sync.dma_start(out=wq[KT], in_=wqa[D : D + 1, :])
            nc.sync.dma_start(out=wv[KT], in_=wva[D : D + 1, :])

            # qT[d, i] : lhsT = wqa[:, d-tile], rhs = xTa[:, i-chunk]
            with tc.tile_pool(name="qps", bufs=2, space="PSUM") as qps:
                for dt in range(4):
                    ps = qps.tile([128, N], f32, tag="qproj")
                    dsl = slice(dt * 128, (dt + 1) * 128)
                    for k in range(KT + 1):
                        for nck in range(4):
                            nsl = slice(nck * 512, (nck + 1) * 512)
                            nc.tensor.matmul(ps[:, nsl], lhsT=wq[k][:, dsl],
                                             rhs=xt[k][:, nsl],
                                             start=(k == 0), stop=(k == KT))
                    nc.scalar.copy(qT[dt], ps)

            # v[i, d] : lhsT = xTa[:, i-tile], rhs = wva ; scatter into v_aug
            with tc.tile_pool(name="vps", bufs=4, space="PSUM") as vps:
                for it in range(JT):
                    ps = vps.tile([128, DL], f32, tag="vproj")
                    isl = slice(it * 128, (it + 1) * 128)
                    for k in range(KT + 1):
                        nc.tensor.matmul(ps, lhsT=xt[k][:, isl], rhs=wv[k],
                                         start=(k == 0), stop=(k == KT))
                    src = ps.rearrange("p (h w) -> p h w", w=64)
                    dst = v_aug[:, it, :].rearrange("p (h w) -> p h w", w=65)
                    nc.scalar.copy(dst[:, :, 0:64], src)

        # allocated after the projection pool closes so phase-1 SBUF peak
        # (xt/wq/wv tiles) and these never coexist in the address map
        p2 = tc.alloc_tile_pool(name="persist2", bufs=1)
        ot = [p2.tile([128, N], bf16, tag=f"ot{t}", name=f"ot{t}")
              for t in range(4)]
        wo_sb = [p2.tile([128, D], bf16, tag=f"wo{t}", name=f"wo{t}")
                 for t in range(4)]
        for t in range(4):
            nc.sync.dma_start(out=wo_sb[t], in_=wo_ap[t * 128 : (t + 1) * 128, :])

        # ---- phase 2a: AA diag pass:  aa[:, h*JT+jt] = -s * ||q_j||^2 ----
        # head pair (2k, 2k+1) lives in row-halves 0:64 / 64:128 of qT[k];
        # interleaving the two K=64 matmuls makes them run concurrently in
        # opposite PE row-half tiles.
        with tc.tile_pool(name="dps", bufs=4, space="PSUM") as dps, \
             tc.tile_pool(name="dsb", bufs=4) as dsb:
            for hp in range(NH // 2):
                dt = hp
                for jq in range(JT // 4):
                    pss = []
                    for half in range(2):
                        rows = slice(half * 64, half * 64 + 64)
                        ps = dps.tile([128, 512], f32, tag="diag")
                        for c in range(4):
                            jt = jq * 4 + c
                            jsl = slice(jt * 128, (jt + 1) * 128)
                            nc.tensor.matmul(ps[:, c * 128 : (c + 1) * 128],
                                             lhsT=qT[dt][rows, jsl],
                                             rhs=qT[dt][rows, jsl],
                                             start=True, stop=True)
                        pss.append(ps)
                    for half in range(2):
                        h = 2 * hp + half
                        col = h * JT + jq * 4
                        sc = dsb.tile([128, 4, 128], f32, tag="dsc")
                        nc.vector.tensor_mul(
                            sc, pss[half].rearrange("p (c w) -> p c w", w=128),
                            ident4)
                        nc.vector.reduce_sum(out=aa[:, col : col + 4], in_=sc,
                                             axis=X)

        # ---- phase 2b: attention, one head pair at a time ----
        with tc.tile_pool(name="sps", bufs=2, space="PSUM") as sps, \
             tc.tile_pool(name="nps", bufs=2, space="PSUM") as nps, \
             tc.tile_pool(name="gp", bufs=6) as gp, \
             tc.tile_pool(name="nrm", bufs=3) as nrm:
            for hp in range(NH // 2):
                dt = hp
                hs = (2 * hp, 2 * hp + 1)
                for ic in range(IC):
                    i0 = ic * ICW
                    nm = [nps.tile([65, ICW], f32, tag="num", name=f"num{half}")
                          for half in range(2)]
                    gs = [[None] * JT for _ in range(2)]
                    for jt in range(JT):
                        jsl = slice(jt * 128, (jt + 1) * 128)
                        sp = []
                        for half in range(2):
                            rows = slice(half * 64, half * 64 + 64)
                            s = sps.tile([128, ICW], f32, tag="scores")
                            for q in range(2):
                                qsl = slice(q * 512, (q + 1) * 512)
                                nc.tensor.matmul(
                                    s[:, qsl], lhsT=qT[dt][rows, jsl],
                                    rhs=qT[dt][rows, i0 + q * 512 : i0 + (q + 1) * 512],
                                    start=True, stop=True)
                            sp.append(s)
                        for half in range(2):
                            g = gp.tile([128, ICW], bf16, tag="gtile")
                            col = hs[half] * JT + jt
                            nc.scalar.activation(out=g, in_=sp[half], func=EXP,
                                                 bias=aa[:, col : col + 1],
                                                 scale=2.0 * SCALE)
                            gs[half][jt] = g
                        # one-step software skew: num(jt-1) after S(jt)/exp(jt)
                        if jt > 0:
                            for half in range(2):
                                vsl = slice(hs[half] * 65, (hs[half] + 1) * 65)
                                for q in range(2):
                                    qsl = slice(q * 512, (q + 1) * 512)
                                    nc.tensor.matmul(
                                        nm[half][:, qsl],
                                        lhsT=v_aug[:, jt - 1, vsl],
                                        rhs=gs[half][jt - 1][:, qsl],
                                        start=(jt == 1), stop=False)
                    for half in range(2):
                        vsl = slice(hs[half] * 65, (hs[half] + 1) * 65)
                        for q in range(2):
                            qsl = slice(q * 512, (q + 1) * 512)
                            nc.tensor.matmul(nm[half][:, qsl],
                                             lhsT=v_aug[:, JT - 1, vsl],
                                             rhs=gs[half][JT - 1][:, qsl],
                                             start=False, stop=True)
                    # normalize: ot[rows, i0:i0+ICW] = nm[0:64] / nm[64].
                    # Copy PSUM->SBUF first so the nm banks free up for the
                    # next unit while the slow reciprocal runs off-path.
                    for half in range(2):
                        rows = slice(half * 64, half * 64 + 64)
                        nmc = nrm.tile([65, ICW], f32, tag="nmc", name="nmc")
                        nc.vector.tensor_copy(nmc, nm[half])
                        rd = nrm.tile([1, ICW], f32, tag="rden", name="rden")
                        nc.vector.reciprocal(rd, nmc[64:65, :])
                        rdb = nrm.tile([64, ICW], f32, tag="rdenb", name="rdenb")
                        nc.gpsimd.partition_broadcast(rdb, rd)
                        nc.vector.tensor_mul(ot[dt][rows, i0 : i0 + ICW],
                                             nmc[0:64, :], rdb)

        # ---- phase 3: output projection ----
        with tc.tile_pool(name="ops", bufs=3, space="PSUM") as ops, \
             tc.tile_pool(name="osb", bufs=3) as osb:
            for it in range(JT):
                isl = slice(it * 128, (it + 1) * 128)
                ps = ops.tile([128, 1024], f32, tag="oproj")
                for ock in range(2):
                    osl = slice(ock * 512, (ock + 1) * 512)
                    for dlt in range(4):
                        nc.tensor.matmul(ps[:, osl], lhsT=ot[dlt][:, isl],
                                         rhs=wo_sb[dlt][:, osl],
                                         start=(dlt == 0), stop=(dlt == 3))
                ob = osb.tile([128, 1024], bf16, tag="obuf", name="obuf")
                nc.scalar.copy(ob, ps)
                nc.sync.dma_start(out=part[isl, :], in_=ob)

        p2.release()

    nc.compile()
    return nc


def _get_nc():
    if "nc" not in _CACHE:
        _CACHE["nc"] = _build_nc()
    return _CACHE["nc"]


def make_in_maps(x, Wq, bq, Wv, bv, Wo, bo):
    import ml_dtypes
    bf16 = ml_dtypes.bfloat16
    x = np.asarray(x, dtype=np.float32)
    Wq = np.asarray(Wq, dtype=np.float32)
    bq = np.asarray(bq, dtype=np.float32)
    Wv = np.asarray(Wv, dtype=np.float32)
    bv = np.asarray(bv, dtype=np.float32)
    Wo = np.asarray(Wo, dtype=np.float32)
    in_maps = []
    for c in range(NCORES):
        b, g = divmod(c, 2)
        gsl = slice(g * DL, (g + 1) * DL)
        xTa = np.concatenate([np.ascontiguousarray(x[b].T),
                              np.ones((1, N), np.float32)], axis=0)
        wqa = np.concatenate([Wq[:, gsl], bq[gsl][None, :]], axis=0)
        wva = np.concatenate([Wv[:, gsl], bv[gsl][None, :]], axis=0)
        in_maps.append({
            "xTa": np.ascontiguousarray(xTa).astype(bf16),
            "wqa": np.ascontiguousarray(wqa).astype(bf16),
            "wva": np.ascontiguousarray(wva).astype(bf16),
            "wo": np.ascontiguousarray(Wo[gsl, :]).astype(bf16),
        })
    return in_maps


def combine_parts(parts, bo):
    bo = np.asarray(bo, dtype=np.float32)
    out = np.empty((B, N, D), np.float32)
    for b in range(B):
        out[b] = np.asarray(parts[2 * b], np.float32) \
            + np.asarray(parts[2 * b + 1], np.float32) + bo
    return out


def kernel(x, Wq, bq, Wv, bv, Wo, bo):
    from concourse.bass_utils import run_bass_kernel_spmd

    nc = _get_nc()
    in_maps = make_in_maps(x, Wq, bq, Wv, bv, Wo, bo)
    res = run_bass_kernel_spmd(nc, in_maps, core_ids=list(range(NCORES)))
    parts = [r["part"] for r in res.results]
    return combine_parts(parts, bo)


# revision 20
# speedup vs baseline: 1.3122x; 1.0781x over previous
"""L2 self-attention (q==k) Bass/Tile kernel for 8 TRN2 NeuronCores.

Sharding: core c = 2*b + g handles batch b and head-group g (8 of 16 heads).
Each core computes the partial output  attn_out_bg @ Wo[g*512:(g+1)*512, :].
Host sums the two partials per batch and adds bo.

Math per head (s = DIM_HEAD**-0.5):
  sim_ij = -s*||q_i - q_j||^2 = 2s*AB_ij - s*AA_i - s*AA_j
  softmax rows are invariant to the per-row constant  -s*AA_i, so
  P_ij = exp(2s*AB_ij - s*AA_j) / sum_j exp(2s*AB_ij - s*AA_j)
  Exponent is <= 0 + bounded (2ab <= a^2+b^2), no max-subtraction needed.
  out = P @ v ; den folded in as a 65th (ones) column of v.

v2: bf16 operands everywhere (halves DMA + enables FWL weight loads),
head-pair interleaving so the two 64-row S matmuls of a dt-tile run
concurrently in opposite PE row-halves, fused AA diag reduction, fast
reciprocal, and output DMA'd straight from PSUM.
"""

import numpy as np

B, N, D = 4, 2048, 1024
HEADS, DIM_HEAD = 16, 64
INNER = HEADS * DIM_HEAD
SCALE = DIM_HEAD ** -0.5

NCORES = 8
NH = 8            # heads per core
DL = NH * DIM_HEAD  # 512 local inner dims
KT = D // 128     # 8 full contraction tiles for projections
JT = N // 128     # 16 key tiles
IC = 2            # i-chunks of 1024 columns
ICW = N // IC     # 1024

_CACHE = {}


def _build_nc():
    import concourse.bacc as bacc
    import concourse.bass as bass
    import concourse.mybir as mybir
    import concourse.tile as tile
    from concourse.masks import make_identity

    f32 = mybir.dt.float32
    bf16 = mybir.dt.bfloat16
    X = mybir.AxisListType.X
    EXP = mybir.ActivationFunctionType.Exp
    MULT = mybir.AluOpType.mult
    ADD = mybir.AluOpType.add

    nc = bacc.Bacc("TRN2", target_bir_lowering=False, debug=False,
                   num_devices=NCORES)

    xTa_d = nc.dram_tensor("xTa", [D + 1, N], bf16, kind="ExternalInput")
    wqa_d = nc.dram_tensor("wqa", [D + 1, DL], bf16, kind="ExternalInput")
    wva_d = nc.dram_tensor("wva", [D + 1, DL], bf16, kind="ExternalInput")
    wo_d = nc.dram_tensor("wo", [DL, D], bf16, kind="ExternalInput")
    part_d = nc.dram_tensor("part", [N, D], bf16, kind="ExternalOutput")
    xTa = xTa_d.ap()
    wqa = wqa_d.ap()
    wva = wva_d.ap()
    wo_ap = wo_d.ap()
    part = part_d.ap()

    with tile.TileContext(nc) as tc, \
         tc.tile_pool(name="persist", bufs=1) as persist:
        # ---- persistent tensors (whole-kernel lifetime) ----
        qT = [persist.tile([128, N], bf16, tag=f"qT{t}", name=f"qT{t}")
              for t in range(4)]
        v_aug = persist.tile([128, JT, NH * 65], bf16, tag="v_aug", name="v_aug")
        aa = persist.tile([128, NH * JT], f32, tag="aa", name="aa")
        ident4 = persist.tile([128, 4, 128], f32, tag="id4", name="ident4")

        ones1 = persist.tile([128, JT, 1], bf16, tag="ones1", name="ones1")
        for c in range(4):
            make_identity(nc, ident4[:, c, :])
        nc.vector.tensor_scalar_mul(ident4, ident4, -SCALE)
        nc.vector.memset(ones1, 1.0)
        for h in range(NH):
            nc.vector.tensor_copy(v_aug[:, :, h * 65 + 64 : h * 65 + 65], ones1)

        # ---- phase 1: projections ----
        with tc.tile_pool(name="pin", bufs=1) as pin:
            xt = [pin.tile([128, N], bf16, tag=f"xt{k}", name=f"xt{k}") for k in range(KT)]
            xt.append(pin.tile([1, N], bf16, tag="xt_ones", name="xt_ones"))
            wq = [pin.tile([128, DL], bf16, tag=f"wq{k}", name=f"wq{k}") for k in range(KT)]
            wq.append(pin.tile([1, DL], bf16, tag="wq_b", name="wq_b"))
            wv = [pin.tile([128, DL], bf16, tag=f"wv{k}", name=f"wv{k}") for k in range(KT)]
            wv.append(pin.tile([1, DL], bf16, tag="wv_b", name="wv_b"))
            for k in range(KT):
                sl = slice(k * 128, (k + 1) * 128)
                nc.sync.dma_start(out=xt[k], in_=xTa[sl, :])
                nc.sync.dma_start(out=wq[k], in_=wqa[sl, :])
            nc.sync.dma_start(out=xt[KT], in_=xTa[D : D + 1, :])
            nc.sync.dma_start(out=wq[KT], in_=wqa[D : D + 1, :])
            for k in range(KT):
                nc.sync.dma_start(out=wv[k], in_=wva[k * 128 : (k + 1) * 128, :])
            nc.sync.dma_start(out=wv[KT], in_=wva[D : D + 1, :])

            # qT[d, i] : lhsT = wqa[:, d-tile], rhs = xTa[:, i-chunk]
            with tc.tile_pool(name="qps", bufs=2, space="PSUM") as qps:
                for dt in range(4):
                    ps = qps.tile([128, N], f32, tag="qproj")
                    dsl = slice(dt * 128, (dt + 1) * 128)
                    for k in range(KT + 1):
                        for nck in range(4):
                            nsl = slice(nck * 512, (nck + 1) * 512)
                            nc.tensor.matmul(ps[:, nsl], lhsT=wq[k][:, dsl],
                                             rhs=xt[k][:, nsl],
                                             start=(k == 0), stop=(k == KT))
                    nc.scalar.copy(qT[dt], ps)

            # v[i, d] : lhsT = xTa[:, i-tile], rhs = wva ; scatter into
            # v_aug. The AA diag pass (grams of qT row-halves; head pair
            # 2k/2k+1 runs concurrently in opposite PE row-half quadrants)
            # interleaves 1:1 with the vproj tiles: separate PSUM pools,
            # AA's reduce runs on DVE while vproj streams on PE.
            with tc.tile_pool(name="vps", bufs=4, space="PSUM") as vps, \
                 tc.tile_pool(name="dps", bufs=4, space="PSUM") as dps, \
                 tc.tile_pool(name="dsb", bufs=4) as dsb:
                for it in range(JT):
                    ps = vps.tile([128, DL], f32, tag="vproj")
                    isl = slice(it * 128, (it + 1) * 128)
                    for k in range(KT + 1):
                        nc.tensor.matmul(ps, lhsT=xt[k][:, isl], rhs=wv[k],
                                         start=(k == 0), stop=(k == KT))
                    vsrc = ps.rearrange("p (h w) -> p h w", w=64)
                    dst = v_aug[:, it, :].rearrange("p (h w) -> p h w", w=65)
                    nc.scalar.copy(dst[:, :, 0:64], vsrc)
                    hp, jq = divmod(it, 4)
                    dt = hp
                    pss = []
                    for half in range(2):
                        rows = slice(half * 64, half * 64 + 64)
                        gm = dps.tile([128, 512], f32, tag="diag")
                        for c in range(4):
                            jt = jq * 4 + c
                            jsl = slice(jt * 128, (jt + 1) * 128)
                            nc.tensor.matmul(gm[:, c * 128 : (c + 1) * 128],
                                             lhsT=qT[dt][rows, jsl],
                                             rhs=qT[dt][rows, jsl],
                                             start=True, stop=True)
                        pss.append(gm)
                    for half in range(2):
                        h = 2 * hp + half
                        col = h * JT + jq * 4
                        sc = dsb.tile([128, 4, 128], f32, tag="dsc")
                        nc.vector.tensor_mul(
                            sc, pss[half].rearrange("p (c w) -> p c w", w=128),
                            ident4)
                        nc.vector.reduce_sum(out=aa[:, col : col + 4], in_=sc,
                                             axis=X)

        # allocated after the projection pool closes so phase-1 SBUF peak
        # (xt/wq/wv tiles) and these never coexist in the address map
        p2 = tc.alloc_tile_pool(name="persist2", bufs=1)
        ot = [p2.tile([128, N], bf16, tag=f"ot{t}", name=f"ot{t}")
              for t in range(4)]
        wo_sb = [p2.tile([128, D], bf16, tag=f"wo{t}", name=f"wo{t}")
                 for t in range(4)]
        for t in range(4):
            nc.sync.dma_start(out=wo_sb[t], in_=wo_ap[t * 128 : (t + 1) * 128, :])

        # ---- phase 2b: attention, one head pair at a time ----
        with tc.tile_pool(name="sps", bufs=2, space="PSUM") as sps, \
             tc.tile_pool(name="nps", bufs=2, space="PSUM") as nps, \
             tc.tile_pool(name="gp", bufs=6) as gp, \
             tc.tile_pool(name="nrm", bufs=3) as nrm:
            for hp in range(NH // 2):
                dt = hp
                hs = (2 * hp, 2 * hp + 1)
                for ic in range(IC):
                    i0 = ic * ICW
                    nm = [nps.tile([65, ICW], f32, tag="num", name=f"num{half}")
                          for half in range(2)]
                    gs = [[None] * JT for _ in range(2)]
                    for jt in range(JT):
                        jsl = slice(jt * 128, (jt + 1) * 128)
                        sp = []
                        for half in range(2):
                            rows = slice(half * 64, half * 64 + 64)
                            s = sps.tile([128, ICW], f32, tag="scores")
                            for q in range(2):
                                qsl = slice(q * 512, (q + 1) * 512)
                                nc.tensor.matmul(
                                    s[:, qsl], lhsT=qT[dt][rows, jsl],
                                    rhs=qT[dt][rows, i0 + q * 512 : i0 + (q + 1) * 512],
                                    start=True, stop=True)
                            sp.append(s)
                        for half in range(2):
                            g = gp.tile([128, ICW], bf16, tag="gtile")
                            col = hs[half] * JT + jt
                            nc.scalar.activation(out=g, in_=sp[half], func=EXP,
                                                 bias=aa[:, col : col + 1],
                                                 scale=2.0 * SCALE)
                            gs[half][jt] = g
                        # one-step software skew: num(jt-1) after S(jt)/exp(jt)
                        if jt > 0:
                            for half in range(2):
                                vsl = slice(hs[half] * 65, (hs[half] + 1) * 65)
                                for q in range(2):
                                    qsl = slice(q * 512, (q + 1) * 512)
                                    nc.tensor.matmul(
                                        nm[half][:, qsl],
                                        lhsT=v_aug[:, jt - 1, vsl],
                                        rhs=gs[half][jt - 1][:, qsl],
                                        start=(jt == 1), stop=False)
                    for half in range(2):
                        vsl = slice(hs[half] * 65, (hs[half] + 1) * 65)
                        for q in range(2):
                            qsl = slice(q * 512, (q + 1) * 512)
                            nc.tensor.matmul(nm[half][:, qsl],
                                             lhsT=v_aug[:, JT - 1, vsl],
                                             rhs=gs[half][JT - 1][:, qsl],
                                             start=False, stop=True)
                    # normalize: ot[rows, i0:i0+ICW] = nm[0:64] / nm[64].
                    # Copy PSUM->SBUF first so the nm banks free up for the
                    # next unit while the slow reciprocal runs off-path.
                    for half in range(2):
                        rows = slice(half * 64, half * 64 + 64)
                        nmc = nrm.tile([65, ICW], f32, tag="nmc", name="nmc")
                        nc.vector.tensor_copy(nmc, nm[half])
                        rd = nrm.tile([1, ICW], f32, tag="rden", name="rden")
                        nc.vector.reciprocal(rd, nmc[64:65, :])
                        rdb = nrm.tile([64, ICW], f32, tag="rdenb", name="rdenb")
                        nc.gpsimd.partition_broadcast(rdb, rd)
                        nc.vector.tensor_mul(ot[dt][rows, i0 : i0 + ICW],
                                             nmc[0:64, :], rdb)

        # ---- phase 3: output projection ----
        with tc.tile_pool(name="ops", bufs=3, space="PSUM") as ops, \
             tc.tile_pool(name="osb", bufs=3) as osb:
            for it in range(JT):
                isl = slice(it * 128, (it + 1) * 128)
                ps = ops.tile([128, 1024], f32, tag="oproj")
                for ock in range(2):
                    osl = slice(ock * 512, (ock + 1) * 512)
                    for dlt in range(4):
                        nc.tensor.matmul(ps[:, osl], lhsT=ot[dlt][:, isl],
                                         rhs=wo_sb[dlt][:, osl],
                                         start=(dlt == 0), stop=(dlt == 3))
                ob = osb.tile([128, 1024], bf16, tag="obuf", name="obuf")
                nc.scalar.copy(ob, ps)
                nc.sync.dma_start(out=part[isl, :], in_=ob)

        p2.release()

    nc.compile()
    return nc


def _get_nc():
    if "nc" not in _CACHE:
        _CACHE["nc"] = _build_nc()
    return _CACHE["nc"]


def make_in_maps(x, Wq, bq, Wv, bv, Wo, bo):
    import ml_dtypes
    bf16 = ml_dtypes.bfloat16
    x = np.asarray(x, dtype=np.float32)
    Wq = np.asarray(Wq, dtype=np.float32)
    bq = np.asarray(bq, dtype=np.float32)
    Wv = np.asarray(Wv, dtype=np.float32)
    bv = np.asarray(bv, dtype=np.float32)
    Wo = np.asarray(Wo, dtype=np.float32)
    in_maps = []
    for c in range(NCORES):
        b, g = divmod(c, 2)
        gsl = slice(g * DL, (g + 1) * DL)
        xTa = np.concatenate([np.ascontiguousarray(x[b].T),
                              np.ones((1, N), np.float32)], axis=0)
        wqa = np.concatenate([Wq[:, gsl], bq[gsl][None, :]], axis=0)
        wva = np.concatenate([Wv[:, gsl], bv[gsl][None, :]], axis=0)
        in_maps.append({
            "xTa": np.ascontiguousarray(xTa).astype(bf16),
            "wqa": np.ascontiguousarray(wqa).astype(bf16),
            "wva": np.ascontiguousarray(wva).astype(bf16),
            "wo": np.ascontiguousarray(Wo[gsl, :]).astype(bf16),
        })
    return in_maps


def combine_parts(parts, bo):
    bo = np.asarray(bo, dtype=np.float32)
    out = np.empty((B, N, D), np.float32)
    for b in range(B):
        out[b] = np.asarray(parts[2 * b], np.float32) \
            + np.asarray(parts[2 * b + 1], np.float32) + bo
    return out


def kernel(x, Wq, bq, Wv, bv, Wo, bo):
    from concourse.bass_utils import run_bass_kernel_spmd

    nc = _get_nc()
    in_maps = make_in_maps(x, Wq, bq, Wv, bv, Wo, bo)
    res = run_bass_kernel_spmd(nc, in_maps, core_ids=list(range(NCORES)))
    parts = [r["part"] for r in res.results]
    return combine_parts(parts, bo)


# revision 22
# speedup vs baseline: 1.3194x; 1.0055x over previous
"""L2 self-attention (q==k) Bass/Tile kernel for 8 TRN2 NeuronCores.

Sharding: core c = 2*b + g handles batch b and head-group g (8 of 16 heads).
Each core computes the partial output  attn_out_bg @ Wo[g*512:(g+1)*512, :].
Host sums the two partials per batch and adds bo.

Math per head (s = DIM_HEAD**-0.5):
  sim_ij = -s*||q_i - q_j||^2 = 2s*AB_ij - s*AA_i - s*AA_j
  softmax rows are invariant to the per-row constant  -s*AA_i, so
  P_ij = exp(2s*AB_ij - s*AA_j) / sum_j exp(2s*AB_ij - s*AA_j)
  Exponent is <= 0 + bounded (2ab <= a^2+b^2), no max-subtraction needed.
  out = P @ v ; den folded in as a 65th (ones) column of v.

v2: bf16 operands everywhere (halves DMA + enables FWL weight loads),
head-pair interleaving so the two 64-row S matmuls of a dt-tile run
concurrently in opposite PE row-halves, fused AA diag reduction, fast
reciprocal, and output DMA'd straight from PSUM.
"""

import numpy as np

B, N, D = 4, 2048, 1024
HEADS, DIM_HEAD = 16, 64
INNER = HEADS * DIM_HEAD
SCALE = DIM_HEAD ** -0.5

NCORES = 8
NH = 8            # heads per core
DL = NH * DIM_HEAD  # 512 local inner dims
KT = D // 128     # 8 full contraction tiles for projections
JT = N // 128     # 16 key tiles
IC = 2            # i-chunks of 1024 columns
ICW = N // IC     # 1024

_CACHE = {}


def _build_nc():
    import concourse.bacc as bacc
    import concourse.bass as bass
    import concourse.mybir as mybir
    import concourse.tile as tile
    from concourse.masks import make_identity

    f32 = mybir.dt.float32
    bf16 = mybir.dt.bfloat16
    X = mybir.AxisListType.X
    EXP = mybir.ActivationFunctionType.Exp
    MULT = mybir.AluOpType.mult
    ADD = mybir.AluOpType.add

    nc = bacc.Bacc("TRN2", target_bir_lowering=False, debug=False,
                   num_devices=NCORES)

    xTa_d = nc.dram_tensor("xTa", [D + 1, N], bf16, kind="ExternalInput")
    wqa_d = nc.dram_tensor("wqa", [D + 1, DL], bf16, kind="ExternalInput")
    wva_d = nc.dram_tensor("wva", [D + 1, DL], bf16, kind="ExternalInput")
    wo_d = nc.dram_tensor("wo", [DL, D], bf16, kind="ExternalInput")
    part_d = nc.dram_tensor("part", [N, D], bf16, kind="ExternalOutput")
    xTa = xTa_d.ap()
    wqa = wqa_d.ap()
    wva = wva_d.ap()
    wo_ap = wo_d.ap()
    part = part_d.ap()

    with tile.TileContext(nc) as tc, \
         tc.tile_pool(name="persist", bufs=1) as persist:
        # ---- persistent tensors (whole-kernel lifetime) ----
        qT = [persist.tile([128, N], bf16, tag=f"qT{t}", name=f"qT{t}")
              for t in range(4)]
        v_aug = persist.tile([128, JT, NH * 65], bf16, tag="v_aug", name="v_aug")
        aa = persist.tile([128, NH * JT], f32, tag="aa", name="aa")
        ident4 = persist.tile([128, 4, 128], f32, tag="id4", name="ident4")

        ones1 = persist.tile([128, JT, 1], bf16, tag="ones1", name="ones1")

        # ---- phase 1: projections ----
        with tc.tile_pool(name="pin", bufs=1) as pin:
            xt = [pin.tile([128, N], bf16, tag=f"xt{k}", name=f"xt{k}") for k in range(KT)]
            xt.append(pin.tile([1, N], bf16, tag="xt_ones", name="xt_ones"))
            wq = [pin.tile([128, DL], bf16, tag=f"wq{k}", name=f"wq{k}") for k in range(KT)]
            wq.append(pin.tile([1, DL], bf16, tag="wq_b", name="wq_b"))
            wv = [pin.tile([128, DL], bf16, tag=f"wv{k}", name=f"wv{k}") for k in range(KT)]
            wv.append(pin.tile([1, DL], bf16, tag="wv_b", name="wv_b"))
            for k in range(KT):
                sl = slice(k * 128, (k + 1) * 128)
                nc.sync.dma_start(out=xt[k], in_=xTa[sl, :])
                nc.sync.dma_start(out=wq[k], in_=wqa[sl, :])
            nc.sync.dma_start(out=xt[KT], in_=xTa[D : D + 1, :])
            nc.sync.dma_start(out=wq[KT], in_=wqa[D : D + 1, :])
            for k in range(KT):
                nc.sync.dma_start(out=wv[k], in_=wva[k * 128 : (k + 1) * 128, :])
            nc.sync.dma_start(out=wv[KT], in_=wva[D : D + 1, :])

            for c in range(4):
                make_identity(nc, ident4[:, c, :])
            nc.vector.tensor_scalar_mul(ident4, ident4, -SCALE)
            nc.vector.memset(ones1, 1.0)
            for h in range(NH):
                nc.vector.tensor_copy(v_aug[:, :, h * 65 + 64 : h * 65 + 65],
                                      ones1)

            # qT[d, i] : lhsT = wqa[:, d-tile], rhs = xTa[:, i-chunk]
            with tc.tile_pool(name="qps", bufs=2, space="PSUM") as qps:
                for dt in range(4):
                    ps = qps.tile([128, N], f32, tag="qproj")
                    dsl = slice(dt * 128, (dt + 1) * 128)
                    for k in range(KT + 1):
                        for nck in range(4):
                            nsl = slice(nck * 512, (nck + 1) * 512)
                            nc.tensor.matmul(ps[:, nsl], lhsT=wq[k][:, dsl],
                                             rhs=xt[k][:, nsl],
                                             start=(k == 0), stop=(k == KT))
                    nc.scalar.copy(qT[dt], ps)

            # v[i, d] : lhsT = xTa[:, i-tile], rhs = wva ; scatter into
            # v_aug. The AA diag pass (grams of qT row-halves; head pair
            # 2k/2k+1 runs concurrently in opposite PE row-half quadrants)
            # interleaves 1:1 with the vproj tiles: separate PSUM pools,
            # AA's reduce runs on DVE while vproj streams on PE.
            with tc.tile_pool(name="vps", bufs=4, space="PSUM") as vps, \
                 tc.tile_pool(name="dps", bufs=4, space="PSUM") as dps, \
                 tc.tile_pool(name="dsb", bufs=4) as dsb:
                for it in range(JT):
                    ps = vps.tile([128, DL], f32, tag="vproj")
                    isl = slice(it * 128, (it + 1) * 128)
                    for k in range(KT + 1):
                        nc.tensor.matmul(ps, lhsT=xt[k][:, isl], rhs=wv[k],
                                         start=(k == 0), stop=(k == KT))
                    vsrc = ps.rearrange("p (h w) -> p h w", w=64)
                    dst = v_aug[:, it, :].rearrange("p (h w) -> p h w", w=65)
                    nc.scalar.copy(dst[:, :, 0:64], vsrc)
                    hp, jq = divmod(it, 4)
                    dt = hp
                    pss = []
                    for half in range(2):
                        rows = slice(half * 64, half * 64 + 64)
                        gm = dps.tile([128, 512], f32, tag="diag")
                        for c in range(4):
                            jt = jq * 4 + c
                            jsl = slice(jt * 128, (jt + 1) * 128)
                            nc.tensor.matmul(gm[:, c * 128 : (c + 1) * 128],
                                             lhsT=qT[dt][rows, jsl],
                                             rhs=qT[dt][rows, jsl],
                                             start=True, stop=True)
                        pss.append(gm)
                    for half in range(2):
                        h = 2 * hp + half
                        col = h * JT + jq * 4
                        sc = dsb.tile([128, 4, 128], f32, tag="dsc")
                        nc.vector.tensor_mul(
                            sc, pss[half].rearrange("p (c w) -> p c w", w=128),
                            ident4)
                        nc.vector.reduce_sum(out=aa[:, col : col + 4], in_=sc,
                                             axis=X)

        # allocated after the projection pool closes so phase-1 SBUF peak
        # (xt/wq/wv tiles) and these never coexist in the address map
        p2 = tc.alloc_tile_pool(name="persist2", bufs=1)
        ot = [p2.tile([128, N], bf16, tag=f"ot{t}", name=f"ot{t}")
              for t in range(4)]
        wo_sb = [p2.tile([128, D], bf16, tag=f"wo{t}", name=f"wo{t}")
                 for t in range(4)]
        for t in range(4):
            nc.sync.dma_start(out=wo_sb[t], in_=wo_ap[t * 128 : (t + 1) * 128, :])

        # ---- phase 2b: attention, one head pair at a time ----
        with tc.tile_pool(name="sps", bufs=2, space="PSUM") as sps, \
             tc.tile_pool(name="nps", bufs=2, space="PSUM") as nps, \
             tc.tile_pool(name="gp", bufs=6) as gp, \
             tc.tile_pool(name="nrm", bufs=3) as nrm:
            for hp in range(NH // 2):
                dt = hp
                hs = (2 * hp, 2 * hp + 1)
                for ic in range(IC):
                    i0 = ic * ICW
                    nm = [nps.tile([65, ICW], f32, tag="num", name=f"num{half}")
                          for half in range(2)]
                    gs = [[None] * JT for _ in range(2)]
                    for jt in range(JT):
                        jsl = slice(jt * 128, (jt + 1) * 128)
                        sp = []
                        for half in range(2):
                            rows = slice(half * 64, half * 64 + 64)
                            s = sps.tile([128, ICW], f32, tag="scores")
                            for q in range(2):
                                qsl = slice(q * 512, (q + 1) * 512)
                                nc.tensor.matmul(
                                    s[:, qsl], lhsT=qT[dt][rows, jsl],
                                    rhs=qT[dt][rows, i0 + q * 512 : i0 + (q + 1) * 512],
                                    start=True, stop=True)
                            sp.append(s)
                        for half in range(2):
                            g = gp.tile([128, ICW], bf16, tag="gtile")
                            col = hs[half] * JT + jt
                            nc.scalar.activation(out=g, in_=sp[half], func=EXP,
                                                 bias=aa[:, col : col + 1],
                                                 scale=2.0 * SCALE)
                            gs[half][jt] = g
                        # one-step software skew: num(jt-1) after S(jt)/exp(jt)
                        if jt > 0:
                            for half in range(2):
                                vsl = slice(hs[half] * 65, (hs[half] + 1) * 65)
                                for q in range(2):
                                    qsl = slice(q * 512, (q + 1) * 512)
                                    nc.tensor.matmul(
                                        nm[half][:, qsl],
                                        lhsT=v_aug[:, jt - 1, vsl],
                                        rhs=gs[half][jt - 1][:, qsl],
                                        start=(jt == 1), stop=False)
                    for half in range(2):
                        vsl = slice(hs[half] * 65, (hs[half] + 1) * 65)
                        for q in range(2):
                            qsl = slice(q * 512, (q + 1) * 512)
                            nc.tensor.matmul(nm[half][:, qsl],
                                             lhsT=v_aug[:, JT - 1, vsl],
                                             rhs=gs[half][JT - 1][:, qsl],
                                             start=False, stop=True)
                    # normalize: ot[rows, i0:i0+ICW] = nm[0:64] / nm[64].
                    # Copy PSUM->SBUF first so the nm banks free up for the
                    # next unit while the slow reciprocal runs off-path.
                    for half in range(2):
                        rows = slice(half * 64, half * 64 + 64)
                        nmc = nrm.tile([65, ICW], f32, tag="nmc", name="nmc")
                        nc.vector.tensor_copy(nmc, nm[half])
                        rd = nrm.tile([1, ICW], f32, tag="rden", name="rden")
                        nc.vector.reciprocal(rd, nmc[64:65, :])
                        rdb = nrm.tile([64, ICW], f32, tag="rdenb", name="rdenb")
                        nc.gpsimd.partition_broadcast(rdb, rd)
                        nc.vector.tensor_mul(ot[dt][rows, i0 : i0 + ICW],
                                             nmc[0:64, :], rdb)

        # ---- phase 3: output projection ----
        with tc.tile_pool(name="ops", bufs=3, space="PSUM") as ops, \
             tc.tile_pool(name="osb", bufs=3) as osb:
            for it in range(JT):
                isl = slice(it * 128, (it + 1) * 128)
                ps = ops.tile([128, 1024], f32, tag="oproj")
                for ock in range(2):
                    osl = slice(ock * 512, (ock + 1) * 512)
                    for dlt in range(4):
                        nc.tensor.matmul(ps[:, osl], lhsT=ot[dlt][:, isl],
                                         rhs=wo_sb[dlt][:, osl],
                                         start=(dlt == 0), stop=(dlt == 3))
                ob = osb.tile([128, 1024], bf16, tag="obuf", name="obuf")
                nc.scalar.copy(ob, ps)
                nc.sync.dma_start(out=part[isl, :], in_=ob)

        p2.release()

    nc.compile()
    return nc


def _get_nc():
    if "nc" not in _CACHE:
        _CACHE["nc"] = _build_nc()
    return _CACHE["nc"]


def make_in_maps(x, Wq, bq, Wv, bv, Wo, bo):
    import ml_dtypes
    bf16 = ml_dtypes.bfloat16
    x = np.asarray(x, dtype=np.float32)
    Wq = np.asarray(Wq, dtype=np.float32)
    bq = np.asarray(bq, dtype=np.float32)
    Wv = np.asarray(Wv, dtype=np.float32)
    bv = np.asarray(bv, dtype=np.float32)
    Wo = np.asarray(Wo, dtype=np.float32)
    in_maps = []
    for c in range(NCORES):
        b, g = divmod(c, 2)
        gsl = slice(g * DL, (g + 1) * DL)
        xTa = np.concatenate([np.ascontiguousarray(x[b].T),
                              np.ones((1, N), np.float32)], axis=0)
        wqa = np.concatenate([Wq[:, gsl], bq[gsl][None, :]], axis=0)
        wva = np.concatenate([Wv[:, gsl], bv[gsl][None, :]], axis=0)
        in_maps.append({
            "xTa": np.ascontiguousarray(xTa).astype(bf16),
            "wqa": np.ascontiguousarray(wqa).astype(bf16),
            "wva": np.ascontiguousarray(wva).astype(bf16),
            "wo": np.ascontiguousarray(Wo[gsl, :]).astype(bf16),
        })
    return in_maps


def combine_parts(parts, bo):
    bo = np.asarray(bo, dtype=np.float32)
    out = np.empty((B, N, D), np.float32)
    for b in range(B):
        out[b] = np.asarray(parts[2 * b], np.float32) \
            + np.asarray(parts[2 * b + 1], np.float32) + bo
    return out


def kernel(x, Wq, bq, Wv, bv, Wo, bo):
    from concourse.bass_utils import run_bass_kernel_spmd

    nc = _get_nc()
    in_maps = make_in_maps(x, Wq, bq, Wv, bv, Wo, bo)
    res = run_bass_kernel_spmd(nc, in_maps, core_ids=list(range(NCORES)))
    parts = [r["part"] for r in res.results]
    return combine_parts(parts, bo)
